# revision 23
# baseline (speedup 1.0000x reference)
"""Attention-pooling layer (u=tanh(Y@W+b); scores=u.w; softmax over S; c=alpha^T Y)
on 8 TRN2 NeuronCores, data-parallel over the batch dim (4 batches/core).

Per-core pipeline (all matmuls in float32r, 1 cycle/row):
  - Y resident in SBUF as f32r [128, 64, 512] (gpsimd rounds DMA-staged chunks)
  - per 512-wide s-chunk: PE-transpose 4x4 128x128 blocks -> Y^T; z^T = W^T Y^T
    accumulated over 4 K-slices; ACT tanh(z^T + b) with per-partition bias;
    scores chunk = w^T u^T on PE; PE re-transposes score rows into the
    [128 part, 64 tile] layout
  - lane-parallel softmax via tiny PE broadcasts (max/exp/sum per batch)
  - pass-2: alpha^T Y with zero-interleaved alpha -> c rows 0..3 of one PSUM bank

Self-contained: hardcodes B=32, S=2048, H=512, 8 cores.
"""
import numpy as np

import concourse.bass as bass
import concourse.tile as tile
from concourse import bacc, mybir
from concourse.bass_utils import run_bass_kernel_spmd
from concourse.masks import make_identity

F32 = mybir.dt.float32
F32R = mybir.dt.float32r

N_CORES = 8
B, S, H = 32, 2048, 512
B_LOC = B // N_CORES          # 4 batches per core
ROWS = B_LOC * S              # 8192 rows per core
P = 128
NT = ROWS // P                # 64 s-tiles of [128, 512]
TPB = S // P                  # 16 s-tiles per batch
HB = H // P                   # 4 h-blocks (K slices)
NC_CHUNKS = NT // 4           # 16 s-chunks of 512

_NC_CACHE = None


def build():
    nc = bacc.Bacc("TRN2", target_bir_lowering=False, debug=False,
                   num_devices=N_CORES)

    Y_ext = nc.declare_dram_parameter("Y", [ROWS, H], F32, isOutput=False)
    m_ext = nc.declare_dram_parameter("mask_Y", [ROWS], F32, isOutput=False)
    W_ext = nc.declare_dram_parameter("W", [H, H], F32, isOutput=False)
    b_ext = nc.declare_dram_parameter("b", [H], F32, isOutput=False)
    w_ext = nc.declare_dram_parameter("w", [H], F32, isOutput=False)
    out_ext = nc.declare_dram_parameter("out", [B_LOC, H], F32, isOutput=True)

    with tile.TileContext(nc) as tc:
        with (
            tc.tile_pool(name="ybig", bufs=1) as ybig,
            tc.tile_pool(name="consts", bufs=1) as consts,
            tc.tile_pool(name="stg", bufs=2) as stg,
            tc.tile_pool(name="ytT", bufs=2) as ytT_pool,
            tc.tile_pool(name="uT", bufs=2) as uT_pool,
            tc.tile_pool(name="small", bufs=1) as small,
            tc.tile_pool(name="tp_ps", bufs=2, space="PSUM") as tp_ps,
            tc.tile_pool(name="z_ps", bufs=2, space="PSUM") as z_ps,
            tc.tile_pool(name="sc_ps", bufs=1, space="PSUM") as sc_ps_pool,
            tc.tile_pool(name="acc_ps", bufs=1, space="PSUM") as acc_ps,
            tc.tile_pool(name="tiny_ps", bufs=1, space="PSUM") as tiny_ps,
        ):
            # ---- constants ----
            identity_f = consts.tile([P, P], F32)
            make_identity(nc, identity_f)
            identity = consts.tile([P, P], F32R)
            nc.gpsimd.tensor_copy(identity[:], identity_f[:])
            one_one = consts.tile([1, 1], F32)
            nc.gpsimd.memset(one_one, 1.0)
            ones_row = consts.tile([1, P], F32)
            nc.gpsimd.memset(ones_row, 1.0)
            ones_col = consts.tile([P, 1], F32)
            nc.gpsimd.memset(ones_col, 1.0)
            # batch indicator BI[p, i, j] = 1 if j == i // TPB else 0
            bi = consts.tile([P, NT, B_LOC], F32)
            nc.gpsimd.memset(bi, 0.0)
            for bb in range(B_LOC):
                nc.gpsimd.memset(bi[:, TPB * bb:TPB * (bb + 1), bb:bb + 1], 1.0)

            # ---- parameters ----
            # W_sb[p, hb, db, e] = W[128*hb + p, 128*db + e], rounded to f32r
            W_raw = consts.tile([P, HB, HB, P], F32)
            nc.scalar.dma_start(
                out=W_raw[:],
                in_=W_ext.ap().rearrange("(hb p) (db e) -> p hb db e",
                                         p=P, e=P))
            W_sb = consts.tile([P, HB, HB, P], F32R)
            nc.gpsimd.tensor_copy(W_sb[:], W_raw[:])
            # b_col[p, db] = b[128*db + p]; w_col likewise (w rounded to f32r)
            b_col = consts.tile([P, HB], F32)
            nc.scalar.dma_start(
                out=b_col[:], in_=b_ext.ap().rearrange("(db p) -> p db", p=P))
            w_raw = consts.tile([P, HB], F32)
            nc.scalar.dma_start(
                out=w_raw[:], in_=w_ext.ap().rearrange("(db p) -> p db", p=P))
            w_col = consts.tile([P, HB], F32R)
            nc.gpsimd.tensor_copy(w_col[:], w_raw[:])
            mask_all = consts.tile([P, NT], F32)
            nc.scalar.dma_start(
                out=mask_all[:], in_=m_ext.ap().rearrange("(i p) -> p i", p=P))

            # ---- bulk Y load: y_all[p, i, :] = Y[128*i + p, :], f32r ----
            y_all = ybig.tile([P, NT, H], F32R)
            y_src = Y_ext.ap().rearrange("(i p) h -> p i h", p=P)
            CHUNK = 2
            for k in range(NT // CHUNK):
                eng = nc.sync if k % 2 == 0 else nc.scalar
                ystg = stg.tile([P, CHUNK, H], F32, tag="stg")
                eng.dma_start(out=ystg[:],
                              in_=y_src[:, k * CHUNK:(k + 1) * CHUNK, :])
                nc.gpsimd.tensor_copy(
                    y_all[:, k * CHUNK:(k + 1) * CHUNK, :], ystg[:])

            sccol_ps = acc_ps.tile([P, NT], F32)

            # ---- pass 1 over 16 s-chunks (software-pipelined by one chunk) --
            def emit_transposes(c):
                """PE-transpose chunk c: ytT[:, hb, 128j:128j+128] =
                y_all[:, 4c+j, hb-block]^T, staged through PSUM."""
                ytT = ytT_pool.tile([P, HB, H], F32R, tag="ytT")
                for hb in range(HB):
                    pt = tp_ps.tile([P, H], F32R)
                    for j in range(4):
                        nc.tensor.transpose(
                            pt[:, j * P:(j + 1) * P],
                            y_all[:, 4 * c + j, hb * P:(hb + 1) * P],
                            identity)
                    nc.scalar.copy(ytT[:, hb, :], pt[:])
                return ytT

            def emit_matmuls(c, ytT):
                uT = uT_pool.tile([P, HB, H], F32R, tag="uT")
                for db in range(HB):
                    zp = z_ps.tile([P, H], F32)
                    for hb in range(HB):
                        nc.tensor.matmul(
                            zp[:],
                            lhsT=W_sb[:, hb, db, :],
                            rhs=ytT[:, hb, :],
                            start=(hb == 0), stop=(hb == HB - 1))
                    nc.scalar.activation(uT[:, db, :], zp[:],
                                         mybir.ActivationFunctionType.Tanh,
                                         bias=b_col[:, db:db + 1])
                # scores chunk [1, 512] = w^T @ u^T, accumulated over db
                scp = sc_ps_pool.tile([1, H], F32)
                for db in range(HB):
                    nc.tensor.matmul(
                        scp[:],
                        lhsT=w_col[:, db:db + 1],
                        rhs=uT[:, db, :],
                        start=(db == 0), stop=(db == HB - 1))
                sc_row = small.tile([1, H], F32, tag="sc_row")
                nc.vector.tensor_copy(sc_row[:], scp[:])
                # transpose the 4 row-segments into columns 4c..4c+3
                for j in range(4):
                    nc.tensor.matmul(
                        sccol_ps[:, 4 * c + j:4 * c + j + 1],
                        lhsT=sc_row[0:1, j * P:(j + 1) * P],
                        rhs=one_one[:],
                        start=True, stop=True)

            prev = None
            for c in range(NC_CHUNKS):
                ytT = emit_transposes(c)
                if prev is not None:
                    emit_matmuls(c - 1, prev)
                prev = ytT
            emit_matmuls(NC_CHUNKS - 1, prev)

            scores = small.tile([P, NT], F32)
            nc.vector.tensor_copy(scores[:], sccol_ps[:])

            # ---- masked softmax over each batch's 2048 scores ----
            mb = small.tile([P, NT], F32)
            nc.vector.tensor_scalar(out=mb[:], in0=mask_all[:],
                                    scalar1=1000.0, scalar2=-1000.0,
                                    op0=mybir.AluOpType.mult,
                                    op1=mybir.AluOpType.add)
            nc.vector.tensor_tensor(out=scores[:], in0=scores[:], in1=mb[:],
                                    op=mybir.AluOpType.add)

            M1 = small.tile([P, B_LOC], F32)
            for bb in range(B_LOC):
                nc.vector.tensor_reduce(
                    out=M1[:, bb:bb + 1],
                    in_=scores[:, TPB * bb:TPB * (bb + 1)],
                    axis=mybir.AxisListType.X, op=mybir.AluOpType.max)

            mt_ps = tiny_ps.tile([B_LOC, P], F32, tag="t1")
            nc.tensor.transpose(mt_ps[:], M1[:], identity_f)
            m_t = small.tile([B_LOC, P], F32)
            nc.vector.tensor_copy(m_t[:], mt_ps[:])
            gmx = small.tile([B_LOC, 1], F32)
            nc.vector.tensor_reduce(out=gmx[:], in_=m_t[:],
                                    axis=mybir.AxisListType.X,
                                    op=mybir.AluOpType.max)
            gr_ps = tiny_ps.tile([1, B_LOC], F32, tag="t1")
            nc.tensor.matmul(gr_ps[:], lhsT=gmx[:],
                             rhs=identity_f[0:B_LOC, 0:B_LOC],
                             start=True, stop=True)
            gmx_row = small.tile([1, B_LOC], F32)
            nc.vector.tensor_copy(gmx_row[:], gr_ps[:])
            bias_ps = tiny_ps.tile([P, B_LOC], F32, tag="t1")
            nc.tensor.matmul(bias_ps[:], lhsT=ones_row[:], rhs=gmx_row[:],
                             start=True, stop=True)
            bias_all = small.tile([P, B_LOC], F32)
            nc.scalar.mul(bias_all[:], bias_ps[:], -1.0)

            exp_sc = small.tile([P, NT], F32)
            S1 = small.tile([P, B_LOC], F32)
            for bb in range(B_LOC):
                nc.scalar.activation(
                    exp_sc[:, TPB * bb:TPB * (bb + 1)],
                    scores[:, TPB * bb:TPB * (bb + 1)],
                    mybir.ActivationFunctionType.Exp,
                    bias=bias_all[:, bb:bb + 1],
                    accum_out=S1[:, bb:bb + 1])

            srow_ps = tiny_ps.tile([1, B_LOC], F32, tag="t1")
            nc.tensor.matmul(srow_ps[:], lhsT=ones_col[:], rhs=S1[:],
                             start=True, stop=True)
            s_row = small.tile([1, B_LOC], F32)
            nc.vector.tensor_copy(s_row[:], srow_ps[:])
            r_row = small.tile([1, B_LOC], F32)
            nc.vector.reciprocal(r_row[:], s_row[:])
            # broadcast 1/S to [128, NT]: column i gets 1/S[i // TPB]
            rb_ps = tiny_ps.tile([P, NT], F32, tag="t1")
            nc.tensor.matmul(
                rb_ps[:], lhsT=ones_row[:],
                rhs=r_row[:].unsqueeze(2).to_broadcast((1, B_LOC, TPB)),
                start=True, stop=True)
            rb = small.tile([P, NT], F32)
            nc.vector.tensor_copy(rb[:], rb_ps[:])
            alpha = small.tile([P, NT], F32)
            nc.vector.tensor_tensor(out=alpha[:], in0=exp_sc[:], in1=rb[:],
                                    op=mybir.AluOpType.mult)
            # zero-interleaved alpha: alphaZ[p, i, j] = alpha[p, i] * BI[p,i,j]
            alphaZ = small.tile([P, NT, B_LOC], F32R)
            nc.vector.tensor_tensor(
                out=alphaZ[:],
                in0=alpha[:].unsqueeze(2).to_broadcast((P, NT, B_LOC)),
                in1=bi[:], op=mybir.AluOpType.mult)

            # ---- pass 2: c[b, :] = sum_i alpha[:, i] . Y_tile_i ----
            c_ps = acc_ps.tile([B_LOC, H], F32)
            for i in range(NT):
                nc.tensor.matmul(
                    c_ps[:],
                    lhsT=alphaZ[:, i, :],
                    rhs=y_all[:, i, :],
                    start=(i == 0), stop=(i == NT - 1))

            c_sb = small.tile([B_LOC, H], F32)
            nc.vector.tensor_copy(c_sb[:], c_ps[:])
            nc.sync.dma_start(out=out_ext[:], in_=c_sb[:])

    nc.compile()
    return nc


def _get_nc():
    global _NC_CACHE
    if _NC_CACHE is None:
        _NC_CACHE = build()
    return _NC_CACHE


def _in_maps(Y, mask_Y, W, b, w):
    Y = np.ascontiguousarray(np.asarray(Y, dtype=np.float32))
    mask_Y = np.ascontiguousarray(np.asarray(mask_Y, dtype=np.float32))
    W = np.ascontiguousarray(np.asarray(W, dtype=np.float32))
    b = np.ascontiguousarray(np.asarray(b, dtype=np.float32))
    w = np.ascontiguousarray(np.asarray(w, dtype=np.float32))
    maps = []
    for c in range(N_CORES):
        ys = np.ascontiguousarray(
            Y[c * B_LOC:(c + 1) * B_LOC].reshape(ROWS, H))
        ms = np.ascontiguousarray(
            mask_Y[c * B_LOC:(c + 1) * B_LOC].reshape(ROWS))
        maps.append({"Y": ys, "mask_Y": ms, "W": W, "b": b, "w": w})
    return maps


def kernel(Y, mask_Y, W, b, w, _trace=False):
    nc = _get_nc()
    maps = _in_maps(Y, mask_Y, W, b, w)
    res = run_bass_kernel_spmd(nc, maps, core_ids=list(range(N_CORES)),
                               trace=_trace)
    out = np.concatenate(
        [np.asarray(res.results[c]["out"]) for c in range(N_CORES)], axis=0)
    if _trace:
        return out.astype(np.float32), res
    return out.astype(np.float32)


# revision 28
# speedup vs baseline: 1.1489x; 1.1489x over previous
"""Attention-pooling layer (u=tanh(Y@W+b); scores=u.w; softmax over S; c=alpha^T Y)
on 8 TRN2 NeuronCores, data-parallel over the batch dim (4 batches/core).

Per-core pipeline (all matmuls in float32r, 1 cycle/row):
  - Y resident in SBUF as f32r [128, 64, 512] (gpsimd rounds DMA-staged chunks)
  - per 512-wide s-chunk: PE-transpose 4x4 128x128 blocks -> Y^T; z^T = W^T Y^T
    accumulated over 4 K-slices; ACT tanh(z^T + b) with per-partition bias;
    scores chunk = w^T u^T on PE; PE re-transposes score rows into the
    [128 part, 64 tile] layout
  - lane-parallel softmax via tiny PE broadcasts (max/exp/sum per batch)
  - pass-2: alpha^T Y with zero-interleaved alpha -> c rows 0..3 of one PSUM bank

Self-contained: hardcodes B=32, S=2048, H=512, 8 cores.
"""
import numpy as np

import concourse.bass as bass
import concourse.tile as tile
from concourse import bacc, mybir
from concourse.bass_utils import run_bass_kernel_spmd
from concourse.masks import make_identity

F32 = mybir.dt.float32
F32R = mybir.dt.float32r

N_CORES = 8
B, S, H = 32, 2048, 512
B_LOC = B // N_CORES          # 4 batches per core
ROWS = B_LOC * S              # 8192 rows per core
P = 128
NT = ROWS // P                # 64 s-tiles of [128, 512]
TPB = S // P                  # 16 s-tiles per batch
HB = H // P                   # 4 h-blocks (K slices)
NC_CHUNKS = NT // 4           # 16 s-chunks of 512

_NC_CACHE = None


def build():
    nc = bacc.Bacc("TRN2", target_bir_lowering=False, debug=False,
                   num_devices=N_CORES)

    Y_ext = nc.declare_dram_parameter("Y", [ROWS, H], F32, isOutput=False)
    m_ext = nc.declare_dram_parameter("mask_Y", [P, NT], F32, isOutput=False)
    W_ext = nc.declare_dram_parameter("W", [H, H], F32, isOutput=False)
    b_ext = nc.declare_dram_parameter("b", [H], F32, isOutput=False)
    w_ext = nc.declare_dram_parameter("w", [H], F32, isOutput=False)
    out_ext = nc.declare_dram_parameter("out", [B_LOC, H], F32, isOutput=True)

    with tile.TileContext(nc) as tc:
        with (
            tc.tile_pool(name="ybig", bufs=1) as ybig,
            tc.tile_pool(name="consts", bufs=1) as consts,
            tc.tile_pool(name="stg", bufs=2) as stg,
            tc.tile_pool(name="ytT", bufs=2) as ytT_pool,
            tc.tile_pool(name="uT", bufs=2) as uT_pool,
            tc.tile_pool(name="small", bufs=1) as small,
            tc.tile_pool(name="tp_ps", bufs=2, space="PSUM") as tp_ps,
            tc.tile_pool(name="z_ps", bufs=2, space="PSUM") as z_ps,
            tc.tile_pool(name="sc_ps", bufs=1, space="PSUM") as sc_ps_pool,
            tc.tile_pool(name="acc_ps", bufs=1, space="PSUM") as acc_ps,
            tc.tile_pool(name="tiny_ps", bufs=1, space="PSUM") as tiny_ps,
        ):
            # ---- constants ----
            identity_f = consts.tile([P, P], F32)
            make_identity(nc, identity_f)
            identity = consts.tile([P, P], F32R)
            nc.vector.tensor_copy(identity[:], identity_f[:])
            one_one = consts.tile([1, 1], F32)
            nc.gpsimd.memset(one_one, 1.0)
            ones_row = consts.tile([1, P], F32)
            nc.gpsimd.memset(ones_row, 1.0)
            ones_col = consts.tile([P, 1], F32)
            nc.gpsimd.memset(ones_col, 1.0)
            # batch indicator BI[p, i, j] = 1 if j == i // TPB else 0
            bi = consts.tile([P, NT, B_LOC], F32)
            nc.gpsimd.memset(bi, 0.0)
            for bb in range(B_LOC):
                nc.gpsimd.memset(bi[:, TPB * bb:TPB * (bb + 1), bb:bb + 1], 1.0)

            # ---- parameters ----
            # W_sb[p, hb, db, e] = W[128*hb + p, 128*db + e], rounded to f32r
            W_raw = consts.tile([P, HB, HB, P], F32)
            nc.scalar.dma_start(
                out=W_raw[:],
                in_=W_ext.ap().rearrange("(hb p) (db e) -> p hb db e",
                                         p=P, e=P))
            W_sb = consts.tile([P, HB, HB, P], F32R)
            nc.vector.tensor_copy(W_sb[:], W_raw[:])
            # b_col[p, db] = b[128*db + p]; w_col likewise (w rounded to f32r)
            b_col = consts.tile([P, HB], F32)
            nc.scalar.dma_start(
                out=b_col[:], in_=b_ext.ap().rearrange("(db p) -> p db", p=P))
            w_raw = consts.tile([P, HB], F32)
            nc.scalar.dma_start(
                out=w_raw[:], in_=w_ext.ap().rearrange("(db p) -> p db", p=P))
            w_col = consts.tile([P, HB], F32R)
            nc.vector.tensor_copy(w_col[:], w_raw[:])
            # mask arrives host-transposed as [128, NT] (contiguous rows)
            mask_all = consts.tile([P, NT], F32)
            nc.scalar.dma_start(out=mask_all[:], in_=m_ext.ap())

            # ---- bulk Y load: y_all[p, i, :] = Y[128*i + p, :], f32r ----
            y_all = ybig.tile([P, NT, H], F32R)
            y_src = Y_ext.ap().rearrange("(i p) h -> p i h", p=P)
            CHUNK = 2
            for k in range(NT // CHUNK):
                eng = nc.sync if k % 2 == 0 else nc.gpsimd
                ystg = stg.tile([P, CHUNK, H], F32, tag="stg")
                eng.dma_start(out=ystg[:],
                              in_=y_src[:, k * CHUNK:(k + 1) * CHUNK, :])
                nc.vector.tensor_copy(
                    y_all[:, k * CHUNK:(k + 1) * CHUNK, :], ystg[:])

            sccol_ps = acc_ps.tile([P, NT], F32)

            # ---- pass 1 over 16 s-chunks (software-pipelined by one chunk) --
            def emit_transposes(c):
                """PE-transpose chunk c: ytT[:, hb, 128j:128j+128] =
                y_all[:, 4c+j, hb-block]^T, staged through PSUM."""
                ytT = ytT_pool.tile([P, HB, H], F32R, tag="ytT")
                for hb in range(HB):
                    pt = tp_ps.tile([P, H], F32R)
                    for j in range(4):
                        nc.tensor.transpose(
                            pt[:, j * P:(j + 1) * P],
                            y_all[:, 4 * c + j, hb * P:(hb + 1) * P],
                            identity)
                    if hb % 2 == 0:
                        nc.scalar.copy(ytT[:, hb, :], pt[:])
                    else:
                        nc.vector.tensor_copy(ytT[:, hb, :], pt[:])
                return ytT

            def emit_matmuls(c, ytT):
                uT = uT_pool.tile([P, HB, H], F32R, tag="uT")
                for db in range(HB):
                    zp = z_ps.tile([P, H], F32)
                    for hb in range(HB):
                        nc.tensor.matmul(
                            zp[:],
                            lhsT=W_sb[:, hb, db, :],
                            rhs=ytT[:, hb, :],
                            start=(hb == 0), stop=(hb == HB - 1))
                    nc.scalar.activation(uT[:, db, :], zp[:],
                                         mybir.ActivationFunctionType.Tanh,
                                         bias=b_col[:, db:db + 1])
                # scores chunk [1, 512] = w^T @ u^T, accumulated over db
                scp = sc_ps_pool.tile([1, H], F32)
                for db in range(HB):
                    nc.tensor.matmul(
                        scp[:],
                        lhsT=w_col[:, db:db + 1],
                        rhs=uT[:, db, :],
                        start=(db == 0), stop=(db == HB - 1))
                sc_row = small.tile([1, H], F32, tag="sc_row")
                nc.vector.tensor_copy(sc_row[:], scp[:])
                # transpose the 4 row-segments into columns 4c..4c+3
                for j in range(4):
                    nc.tensor.matmul(
                        sccol_ps[:, 4 * c + j:4 * c + j + 1],
                        lhsT=sc_row[0:1, j * P:(j + 1) * P],
                        rhs=one_one[:],
                        start=True, stop=True)

            prev = None
            for c in range(NC_CHUNKS):
                ytT = emit_transposes(c)
                if prev is not None:
                    emit_matmuls(c - 1, prev)
                prev = ytT
            emit_matmuls(NC_CHUNKS - 1, prev)

            scores = small.tile([P, NT], F32)
            nc.vector.tensor_copy(scores[:], sccol_ps[:])

            # ---- masked softmax over each batch's 2048 scores ----
            mb = small.tile([P, NT], F32)
            nc.vector.tensor_scalar(out=mb[:], in0=mask_all[:],
                                    scalar1=1000.0, scalar2=-1000.0,
                                    op0=mybir.AluOpType.mult,
                                    op1=mybir.AluOpType.add)
            nc.vector.tensor_tensor(out=scores[:], in0=scores[:], in1=mb[:],
                                    op=mybir.AluOpType.add)

            M1 = small.tile([P, B_LOC], F32)
            for bb in range(B_LOC):
                nc.vector.tensor_reduce(
                    out=M1[:, bb:bb + 1],
                    in_=scores[:, TPB * bb:TPB * (bb + 1)],
                    axis=mybir.AxisListType.X, op=mybir.AluOpType.max)

            mt_ps = tiny_ps.tile([B_LOC, P], F32, tag="t1")
            nc.tensor.transpose(mt_ps[:], M1[:], identity_f)
            m_t = small.tile([B_LOC, P], F32)
            nc.vector.tensor_copy(m_t[:], mt_ps[:])
            gmx = small.tile([B_LOC, 1], F32)
            nc.vector.tensor_reduce(out=gmx[:], in_=m_t[:],
                                    axis=mybir.AxisListType.X,
                                    op=mybir.AluOpType.max)
            gr_ps = tiny_ps.tile([1, B_LOC], F32, tag="t1")
            nc.tensor.matmul(gr_ps[:], lhsT=gmx[:],
                             rhs=identity_f[0:B_LOC, 0:B_LOC],
                             start=True, stop=True)
            gmx_row = small.tile([1, B_LOC], F32)
            nc.vector.tensor_copy(gmx_row[:], gr_ps[:])
            bias_ps = tiny_ps.tile([P, B_LOC], F32, tag="t1")
            nc.tensor.matmul(bias_ps[:], lhsT=ones_row[:], rhs=gmx_row[:],
                             start=True, stop=True)
            bias_all = small.tile([P, B_LOC], F32)
            nc.scalar.mul(bias_all[:], bias_ps[:], -1.0)

            exp_sc = small.tile([P, NT], F32)
            S1 = small.tile([P, B_LOC], F32)
            for bb in range(B_LOC):
                nc.scalar.activation(
                    exp_sc[:, TPB * bb:TPB * (bb + 1)],
                    scores[:, TPB * bb:TPB * (bb + 1)],
                    mybir.ActivationFunctionType.Exp,
                    bias=bias_all[:, bb:bb + 1],
                    accum_out=S1[:, bb:bb + 1])

            srow_ps = tiny_ps.tile([1, B_LOC], F32, tag="t1")
            nc.tensor.matmul(srow_ps[:], lhsT=ones_col[:], rhs=S1[:],
                             start=True, stop=True)
            s_row = small.tile([1, B_LOC], F32)
            nc.vector.tensor_copy(s_row[:], srow_ps[:])
            r_row = small.tile([1, B_LOC], F32)
            nc.vector.reciprocal(r_row[:], s_row[:])
            # broadcast 1/S to [128, NT]: column i gets 1/S[i // TPB]
            rb_ps = tiny_ps.tile([P, NT], F32, tag="t1")
            nc.tensor.matmul(
                rb_ps[:], lhsT=ones_row[:],
                rhs=r_row[:].unsqueeze(2).to_broadcast((1, B_LOC, TPB)),
                start=True, stop=True)
            rb = small.tile([P, NT], F32)
            nc.vector.tensor_copy(rb[:], rb_ps[:])
            alpha = small.tile([P, NT], F32)
            nc.vector.tensor_tensor(out=alpha[:], in0=exp_sc[:], in1=rb[:],
                                    op=mybir.AluOpType.mult)
            # zero-interleaved alpha: alphaZ[p, i, j] = alpha[p, i] * BI[p,i,j]
            alphaZ = small.tile([P, NT, B_LOC], F32R)
            nc.vector.tensor_tensor(
                out=alphaZ[:],
                in0=alpha[:].unsqueeze(2).to_broadcast((P, NT, B_LOC)),
                in1=bi[:], op=mybir.AluOpType.mult)

            # ---- pass 2: c[b, :] = sum_i alpha[:, i] . Y_tile_i ----
            c_ps = acc_ps.tile([B_LOC, H], F32)
            for i in range(NT):
                nc.tensor.matmul(
                    c_ps[:],
                    lhsT=alphaZ[:, i, :],
                    rhs=y_all[:, i, :],
                    start=(i == 0), stop=(i == NT - 1))

            c_sb = small.tile([B_LOC, H], F32)
            nc.vector.tensor_copy(c_sb[:], c_ps[:])
            nc.sync.dma_start(out=out_ext[:], in_=c_sb[:])

    nc.compile()
    return nc


def _get_nc():
    global _NC_CACHE
    if _NC_CACHE is None:
        _NC_CACHE = build()
    return _NC_CACHE


def _in_maps(Y, mask_Y, W, b, w):
    Y = np.ascontiguousarray(np.asarray(Y, dtype=np.float32))
    mask_Y = np.ascontiguousarray(np.asarray(mask_Y, dtype=np.float32))
    W = np.ascontiguousarray(np.asarray(W, dtype=np.float32))
    b = np.ascontiguousarray(np.asarray(b, dtype=np.float32))
    w = np.ascontiguousarray(np.asarray(w, dtype=np.float32))
    maps = []
    for c in range(N_CORES):
        ys = np.ascontiguousarray(
            Y[c * B_LOC:(c + 1) * B_LOC].reshape(ROWS, H))
        ms = np.ascontiguousarray(
            mask_Y[c * B_LOC:(c + 1) * B_LOC].reshape(NT, P).T)
        maps.append({"Y": ys, "mask_Y": ms, "W": W, "b": b, "w": w})
    return maps


def kernel(Y, mask_Y, W, b, w, _trace=False):
    nc = _get_nc()
    maps = _in_maps(Y, mask_Y, W, b, w)
    res = run_bass_kernel_spmd(nc, maps, core_ids=list(range(N_CORES)),
                               trace=_trace)
    out = np.concatenate(
        [np.asarray(res.results[c]["out"]) for c in range(N_CORES)], axis=0)
    if _trace:
        return out.astype(np.float32), res
    return out.astype(np.float32)


# revision 29
# speedup vs baseline: 1.3287x; 1.1565x over previous
"""Attention-pooling layer (u=tanh(Y@W+b); scores=u.w; softmax over S; c=alpha^T Y)
on 8 TRN2 NeuronCores, data-parallel over the batch dim (4 batches/core).

Per-core pipeline (matmuls in float32r):
  - Y resident in SBUF as f32r [128, 64, 512] (DVE rounds DMA-staged chunks)
  - per 512-wide s-chunk: PE-transpose 16x 128x128 blocks -> Y^T; z^T = W^T Y^T
    (4 K-slices into PSUM); ACT tanh(z^T + b) with per-partition bias;
    scores chunk = w^T u^T on PE; tiny PE transposes land scores in
    [128 part, 64 tile] layout
  - per-batch softmax + pass-2 are interleaved into pass-1: as soon as a
    batch's 4 chunks are scored, its max/exp/sum run on DVE/ACT and its 16
    alpha^T-Y matmuls join one long PSUM accumulation group
  - normalization by 1/sum(exp) is deferred to the final PSUM->SBUF copy

Self-contained: hardcodes B=32, S=2048, H=512, 8 cores.
"""
import numpy as np

import concourse.bass as bass
import concourse.tile as tile
from concourse import bacc, mybir
from concourse.bass_utils import run_bass_kernel_spmd
from concourse.masks import make_identity

F32 = mybir.dt.float32
F32R = mybir.dt.float32r

N_CORES = 8
B, S, H = 32, 2048, 512
B_LOC = B // N_CORES          # 4 batches per core
ROWS = B_LOC * S              # 8192 rows per core
P = 128
NT = ROWS // P                # 64 s-tiles of [128, 512]
TPB = S // P                  # 16 s-tiles per batch
HB = H // P                   # 4 h-blocks (K slices)
NCH = NT // 4                 # 16 s-chunks of 512
CPB = NCH // B_LOC            # 4 chunks per batch

_NC_CACHE = None


def build():
    nc = bacc.Bacc("TRN2", target_bir_lowering=False, debug=False,
                   num_devices=N_CORES)

    Y_ext = nc.declare_dram_parameter("Y", [ROWS, H], F32, isOutput=False)
    m_ext = nc.declare_dram_parameter("mask_Y", [P, NT], F32, isOutput=False)
    W_ext = nc.declare_dram_parameter("W", [H, H], F32, isOutput=False)
    b_ext = nc.declare_dram_parameter("b", [H], F32, isOutput=False)
    w_ext = nc.declare_dram_parameter("w", [H], F32, isOutput=False)
    out_ext = nc.declare_dram_parameter("out", [B_LOC, H], F32, isOutput=True)

    with tile.TileContext(nc) as tc:
        with (
            tc.tile_pool(name="ybig", bufs=1) as ybig,
            tc.tile_pool(name="consts", bufs=1) as consts,
            tc.tile_pool(name="stg", bufs=2) as stg,
            tc.tile_pool(name="ytT", bufs=2) as ytT_pool,
            tc.tile_pool(name="uT", bufs=2) as uT_pool,
            tc.tile_pool(name="small", bufs=1) as small,
            tc.tile_pool(name="sm", bufs=2) as sm_pool,
            tc.tile_pool(name="tp_ps", bufs=2, space="PSUM") as tp_ps,
            tc.tile_pool(name="z_ps", bufs=2, space="PSUM") as z_ps,
            tc.tile_pool(name="sc_ps", bufs=1, space="PSUM") as sc_ps_pool,
            tc.tile_pool(name="acc_ps", bufs=1, space="PSUM") as acc_ps,
            tc.tile_pool(name="tiny_ps", bufs=1, space="PSUM") as tiny_ps,
        ):
            # ---- constants ----
            identity_f = consts.tile([P, P], F32)
            make_identity(nc, identity_f)
            identity = consts.tile([P, P], F32R)
            nc.vector.tensor_copy(identity[:], identity_f[:])
            one_one = consts.tile([1, 1], F32)
            nc.gpsimd.memset(one_one, 1.0)
            ones_row = consts.tile([1, P], F32)
            nc.gpsimd.memset(ones_row, 1.0)
            ones_col = consts.tile([P, 1], F32)
            nc.gpsimd.memset(ones_col, 1.0)
            # batch indicator BI[p, i, j] = 1 if j == i // TPB else 0 (f32r,
            # produced by compute so alphaZ = exp * BI is a legal f32r input)
            bi = consts.tile([P, NT, B_LOC], F32)
            nc.gpsimd.memset(bi, 0.0)
            for bb in range(B_LOC):
                nc.gpsimd.memset(bi[:, TPB * bb:TPB * (bb + 1), bb:bb + 1], 1.0)

            # ---- parameters ----
            W_raw = consts.tile([P, HB, HB, P], F32)
            nc.scalar.dma_start(
                out=W_raw[:],
                in_=W_ext.ap().rearrange("(hb p) (db e) -> p hb db e",
                                         p=P, e=P))
            W_sb = consts.tile([P, HB, HB, P], F32R)
            nc.vector.tensor_copy(W_sb[:], W_raw[:])
            b_col = consts.tile([P, HB], F32)
            nc.scalar.dma_start(
                out=b_col[:], in_=b_ext.ap().rearrange("(db p) -> p db", p=P))
            w_raw = consts.tile([P, HB], F32)
            nc.scalar.dma_start(
                out=w_raw[:], in_=w_ext.ap().rearrange("(db p) -> p db", p=P))
            w_col = consts.tile([P, HB], F32R)
            nc.vector.tensor_copy(w_col[:], w_raw[:])
            # mask arrives host-transposed as [128, NT]; fold to additive bias
            mask_all = consts.tile([P, NT], F32)
            nc.scalar.dma_start(out=mask_all[:], in_=m_ext.ap())
            mbias = consts.tile([P, NT], F32)
            nc.vector.tensor_scalar(out=mbias[:], in0=mask_all[:],
                                    scalar1=1000.0, scalar2=-1000.0,
                                    op0=mybir.AluOpType.mult,
                                    op1=mybir.AluOpType.add)

            # ---- bulk Y load: y_all[p, i, :] = Y[128*i + p, :], f32r ----
            y_all = ybig.tile([P, NT, H], F32R)
            y_src = Y_ext.ap().rearrange("(i p) h -> p i h", p=P)
            CHUNK = 2
            for k in range(NT // CHUNK):
                eng = nc.sync if k % 2 == 0 else nc.gpsimd
                ystg = stg.tile([P, CHUNK, H], F32, tag="stg")
                eng.dma_start(out=ystg[:],
                              in_=y_src[:, k * CHUNK:(k + 1) * CHUNK, :])
                nc.vector.tensor_copy(
                    y_all[:, k * CHUNK:(k + 1) * CHUNK, :], ystg[:])

            sccol_ps = acc_ps.tile([P, NT], F32)
            c_ps = acc_ps.tile([B_LOC, H], F32, tag="c")
            scores = small.tile([P, NT], F32)
            exp_sc = small.tile([P, NT], F32)
            S_row = small.tile([1, B_LOC], F32)

            def emit_transposes(c):
                ytT = ytT_pool.tile([P, HB, H], F32R, tag="ytT")
                for hb in range(HB):
                    pt = tp_ps.tile([P, H], F32R)
                    for j in range(4):
                        nc.tensor.transpose(
                            pt[:, j * P:(j + 1) * P],
                            y_all[:, 4 * c + j, hb * P:(hb + 1) * P],
                            identity)
                    # split the PSUM->SBUF copy across ACT and DVE
                    nc.scalar.copy(ytT[:, hb, 0:H // 2], pt[:, 0:H // 2])
                    nc.vector.tensor_copy(ytT[:, hb, H // 2:H],
                                          pt[:, H // 2:H])
                return ytT

            def emit_matmuls(c, ytT):
                uT = uT_pool.tile([P, HB, H], F32R, tag="uT")
                for db in range(HB):
                    zp = z_ps.tile([P, H], F32)
                    for hb in range(HB):
                        nc.tensor.matmul(
                            zp[:],
                            lhsT=W_sb[:, hb, db, :],
                            rhs=ytT[:, hb, :],
                            start=(hb == 0), stop=(hb == HB - 1))
                    nc.scalar.activation(uT[:, db, :], zp[:],
                                         mybir.ActivationFunctionType.Tanh,
                                         bias=b_col[:, db:db + 1])
                scp = sc_ps_pool.tile([1, H], F32)
                for db in range(HB):
                    nc.tensor.matmul(
                        scp[:],
                        lhsT=w_col[:, db:db + 1],
                        rhs=uT[:, db, :],
                        start=(db == 0), stop=(db == HB - 1))
                sc_row = sm_pool.tile([1, H], F32, tag="sc_row")
                nc.vector.tensor_copy(sc_row[:], scp[:])
                for j in range(4):
                    nc.tensor.matmul(
                        sccol_ps[:, 4 * c + j:4 * c + j + 1],
                        lhsT=sc_row[0:1, j * P:(j + 1) * P],
                        rhs=one_one[:],
                        start=True, stop=True)

            def emit_batch_tail(bb):
                """Softmax for batch bb + its 16 pass-2 matmuls (interleaved
                with the next batch's pass-1 work by the scheduler)."""
                lo, hi = TPB * bb, TPB * (bb + 1)
                nc.vector.tensor_copy(scores[:, lo:hi], sccol_ps[:, lo:hi])
                nc.vector.tensor_tensor(out=scores[:, lo:hi],
                                        in0=scores[:, lo:hi],
                                        in1=mbias[:, lo:hi],
                                        op=mybir.AluOpType.add)
                m1 = sm_pool.tile([P, 1], F32, tag="m1")
                nc.vector.tensor_reduce(out=m1[:], in_=scores[:, lo:hi],
                                        axis=mybir.AxisListType.X,
                                        op=mybir.AluOpType.max)
                m1t_ps = tiny_ps.tile([1, P], F32, tag="t1")
                nc.tensor.matmul(m1t_ps[:], lhsT=m1[:], rhs=identity_f[:],
                                 start=True, stop=True)
                m1t = sm_pool.tile([1, P], F32, tag="m1t")
                nc.vector.tensor_copy(m1t[:], m1t_ps[:])
                mx11 = sm_pool.tile([1, 1], F32, tag="mx11")
                nc.vector.tensor_reduce(out=mx11[:], in_=m1t[:],
                                        axis=mybir.AxisListType.X,
                                        op=mybir.AluOpType.max)
                bia_ps = tiny_ps.tile([P, 1], F32, tag="t1")
                nc.tensor.matmul(bia_ps[:], lhsT=ones_row[:], rhs=mx11[:],
                                 start=True, stop=True)
                bias_b = sm_pool.tile([P, 1], F32, tag="bias_b")
                nc.scalar.mul(bias_b[:], bia_ps[:], -1.0)
                s1 = sm_pool.tile([P, 1], F32, tag="s1")
                nc.scalar.activation(
                    exp_sc[:, lo:hi], scores[:, lo:hi],
                    mybir.ActivationFunctionType.Exp,
                    bias=bias_b[:], accum_out=s1[:])
                sb_ps = tiny_ps.tile([1, 1], F32, tag="t1")
                nc.tensor.matmul(sb_ps[:], lhsT=ones_col[:], rhs=s1[:],
                                 start=True, stop=True)
                nc.vector.tensor_copy(S_row[:, bb:bb + 1], sb_ps[:])
                # zero-interleaved unnormalized alpha for this batch
                aZ = sm_pool.tile([P, TPB, B_LOC], F32R, tag="aZ")
                nc.vector.tensor_tensor(
                    out=aZ[:],
                    in0=exp_sc[:, lo:hi].unsqueeze(2).to_broadcast(
                        (P, TPB, B_LOC)),
                    in1=bi[:, lo:hi, :], op=mybir.AluOpType.mult)
                for t in range(TPB):
                    i = lo + t
                    nc.tensor.matmul(
                        c_ps[:],
                        lhsT=aZ[:, t, :],
                        rhs=y_all[:, i, :],
                        start=(i == 0), stop=(i == NT - 1),
                        skip_group_check=True)

            prev = None
            for c in range(NCH):
                ytT = emit_transposes(c)
                if prev is not None:
                    emit_matmuls(c - 1, prev)
                    if c % CPB == 0:
                        emit_batch_tail(c // CPB - 1)
                prev = ytT
            emit_matmuls(NCH - 1, prev)
            emit_batch_tail(B_LOC - 1)

            # ---- finalize: c[b, :] /= S[b] ----
            r_row = small.tile([1, B_LOC], F32)
            nc.vector.reciprocal(r_row[:], S_row[:])
            rc_ps = tiny_ps.tile([B_LOC, 1], F32, tag="t1")
            nc.tensor.matmul(rc_ps[:], lhsT=r_row[:], rhs=one_one[:],
                             start=True, stop=True)
            r_col = small.tile([B_LOC, 1], F32)
            nc.vector.tensor_copy(r_col[:], rc_ps[:])
            c_sb = small.tile([B_LOC, H], F32)
            nc.vector.tensor_scalar(out=c_sb[:], in0=c_ps[:],
                                    scalar1=r_col[:], scalar2=None,
                                    op0=mybir.AluOpType.mult)
            nc.sync.dma_start(out=out_ext[:], in_=c_sb[:])

    nc.compile()
    return nc


def _get_nc():
    global _NC_CACHE
    if _NC_CACHE is None:
        _NC_CACHE = build()
    return _NC_CACHE


def _in_maps(Y, mask_Y, W, b, w):
    Y = np.ascontiguousarray(np.asarray(Y, dtype=np.float32))
    mask_Y = np.ascontiguousarray(np.asarray(mask_Y, dtype=np.float32))
    W = np.ascontiguousarray(np.asarray(W, dtype=np.float32))
    b = np.ascontiguousarray(np.asarray(b, dtype=np.float32))
    w = np.ascontiguousarray(np.asarray(w, dtype=np.float32))
    maps = []
    for c in range(N_CORES):
        ys = np.ascontiguousarray(
            Y[c * B_LOC:(c + 1) * B_LOC].reshape(ROWS, H))
        ms = np.ascontiguousarray(
            mask_Y[c * B_LOC:(c + 1) * B_LOC].reshape(NT, P).T)
        maps.append({"Y": ys, "mask_Y": ms, "W": W, "b": b, "w": w})
    return maps


def kernel(Y, mask_Y, W, b, w, _trace=False):
    nc = _get_nc()
    maps = _in_maps(Y, mask_Y, W, b, w)
    res = run_bass_kernel_spmd(nc, maps, core_ids=list(range(N_CORES)),
                               trace=_trace)
    out = np.concatenate(
        [np.asarray(res.results[c]["out"]) for c in range(N_CORES)], axis=0)
    if _trace:
        return out.astype(np.float32), res
    return out.astype(np.float32)
